# revision 12
# baseline (speedup 1.0000x reference)
"""Trainium2 Bass kernel for nn_Attention_1322849927460.

Dense transformer block: LN -> qkv -> attention (+ spatial-bias MLP on
attention-weighted coordinate deltas) -> out proj -> gelu -> residual.

Sharding: 8 cores = (2 batches) x (4 sequence quarters). Each core holds
all 8 heads for its 512 query rows and the full 2048-token K/V of its
batch, so no collectives are needed. A host-side roll of the token axis
puts each core's query rows first, letting all cores run an identical
SPMD program (attention is invariant to key-order permutation).

Algebraic structure:
  * delta_full[b,h,i,:] = (attn @ xyz)[b,h,i,:] - xyz[b,i,:] since softmax
    rows sum to one -> the (m,m,3) delta tensor is never formed.
  * softmax denominators come free from augmented V' columns [xyz/32, 1/32];
    one reciprocal + partition-broadcast normalizes the accumulators.  The
    1/32 ones-column also lands the normalized attention output at 32x
    true scale, lifting the fp8 outfin tensor out of e4m3 denormals free.
  * ln_g and the 1/sqrt(dh) q-scale fold into the qkv weights on host.

fp8 strategy (vs the bf16 baseline):
  * q/k projections run as fp8e4 DoubleRow matmuls (the two 128-row DIM
    halves as k-tiles, 0.5 cycles/col); v and the out projection run as
    plain fp8 matmuls (their outputs span 128 partitions, and DoubleRow
    outputs must start at partition 0 - walrus crashes otherwise).
  * AV runs as DoubleRow over adjacent j-tile pairs: stationary
    vaug[128tok, 2tiles, 64] against moving e[128, 2tiles, 512], with the
    4 xyz/ones columns as a second small DoubleRow into a base-0 [4, .]
    accumulator that a tiny DMA relocates to partitions 64:68.
  * spatial-MLP h2 is DoubleRow over kc pairs.  QK and MLP h1 stay bf16
    (PSUM-output-bound; fp8 wouldn't speed them).
  * q/k live in a [64, 8heads, tokens] layout so DoubleRow 64-row outputs
    evacuate without crossing partitions (QK was column-serial anyway).
  * weights quantized with power-of-2 host scales (wqkv x64, spw2 x32,
    wout x64), descaled for free inside evacuation ops / the final ACT.
  * exp alternates: even j-tiles exact on the scalar engine (fp8 out),
    odd j-tiles on the DVE via Schraudolph straight into e4m3 bits
    (i8 = round(x*8/ln2 + 55.66), bitcast).
"""

import os
import sys

for _p in ("/opt/trn_rl_repo",):
    if _p not in sys.path and os.path.isdir(_p):
        sys.path.insert(0, _p)

import ml_dtypes
import numpy as np

import concourse.bass as bass
import concourse.bacc as bacc
import concourse.tile as tile
from concourse import mybir
from concourse.bass_utils import run_bass_kernel_spmd
from concourse.masks import make_identity

F32 = mybir.dt.float32
BF16 = mybir.dt.bfloat16
F8 = mybir.dt.float8e4
I8 = mybir.dt.int8
AF = mybir.ActivationFunctionType
OP = mybir.AluOpType
DR = mybir.MatmulPerfMode.DoubleRow
BF = ml_dtypes.bfloat16
E4 = ml_dtypes.float8_e4m3

DIM = 256
H = 8
DH = 64
INNER = H * DH  # 512
M = 2048  # tokens per batch
TQ = 512  # query tokens per core
NT = M // 128  # 16 token tiles
N_CORES = 8
LN_EPS = 1e-5

# host-side fp8 weight scales (powers of two; descaled on-chip for free)
WQ_S = 64.0      # wqkv scale; descale 2^-6 in the q/k/v evacuations
W2_S = 32.0      # spw2 scale == the outfin x32 target scale
WO_S = 64.0      # wout scale; total descale 2^-11 in the final gelu
OF_S = 32.0      # outfin scale, produced by the 1/32 ones/xyz columns
WQ_INV = 1.0 / WQ_S
YT_INV = 1.0 / (OF_S * WO_S)

# Schraudolph fast exp in e4m3 bit domain:
#   e4m3(x) bits = round(x * 2^3/ln2 + (7*2^3 - 0.34)) viewed as int8.
EXP_A8 = 8.0 / float(np.log(2.0))
EXP_B8 = 56.0 - 0.34


def build_program(has_bqkv: bool, has_spb1: bool, has_spb2: bool):
    nc = bacc.Bacc()

    x_d = nc.dram_tensor("x", [M, DIM], BF16, kind="ExternalInput")
    xyzv_d = nc.dram_tensor("xyzv", [128, NT, 4], F8, kind="ExternalInput")
    xyzt_d = nc.dram_tensor("xyzt", [3, TQ], BF16, kind="ExternalInput")
    featt_d = nc.dram_tensor("featt", [DIM, TQ], F32, kind="ExternalInput")
    wqkv_d = nc.dram_tensor("wqkv", [DIM, 3 * INNER], F8, kind="ExternalInput")
    spw1_d = nc.dram_tensor("spw1", [3, 2 * DIM], BF16, kind="ExternalInput")
    spw2_d = nc.dram_tensor("spw2", [2 * DIM, DH], F8, kind="ExternalInput")
    wout_d = nc.dram_tensor("wout", [64, H, DIM], F8, kind="ExternalInput")
    cf32_d = nc.dram_tensor("cf32", [128, 24], F32, kind="ExternalInput")
    cbf_d = nc.dram_tensor("cbf", [1, TQ + INNER + DH], BF16, kind="ExternalInput")
    out_d = nc.dram_tensor("out", [DIM, TQ], F32, kind="ExternalOutput")

    with tile.TileContext(nc) as tc:
        with (
            tc.tile_pool(name="const", bufs=1) as constp,
            tc.tile_pool(name="big", bufs=1) as bigp,
            tc.tile_pool(name="work", bufs=2) as workp,
        ):
            # ---- DMAs: all on the sync HWDGE queue, critical-path first.
            wqkv_sb = constp.tile([128, 2, 3 * INNER], F8)
            nc.sync.dma_start(
                out=wqkv_sb, in_=wqkv_d[:].rearrange("(cc p) o -> p cc o", p=128)
            )
            x_sb = bigp.tile([128, NT, DIM], BF16)
            xv = x_d[:].rearrange("(n p) c -> p n c", p=128)
            for g in range(4):
                nc.sync.dma_start(
                    out=x_sb[:, 4 * g : 4 * g + 4, :],
                    in_=xv[:, 4 * g : 4 * g + 4, :],
                )
            xyzv_sb = constp.tile([128, NT, 4], F8)
            nc.sync.dma_start(out=xyzv_sb, in_=xyzv_d[:])
            xyzt_sb = constp.tile([3, TQ], BF16)
            nc.sync.dma_start(out=xyzt_sb, in_=xyzt_d[:])
            cbf_sb = constp.tile([1, TQ + INNER + DH], BF16)
            nc.sync.dma_start(out=cbf_sb, in_=cbf_d[:])
            cf32_sb = constp.tile([128, 24], F32)
            nc.sync.dma_start(out=cf32_sb, in_=cf32_d[:])
            spw1_sb = constp.tile([3, 2 * DIM], BF16)
            nc.sync.dma_start(out=spw1_sb, in_=spw1_d[:])
            spw2_sb = constp.tile([128, 4, DH], F8)
            nc.sync.dma_start(
                out=spw2_sb, in_=spw2_d[:].rearrange("(kc p) d -> p kc d", p=128)
            )
            wout_sb = constp.tile([64, H, DIM], F8)
            nc.sync.dma_start(out=wout_sb, in_=wout_d[:])
            featt_sb = constp.tile([128, 2, TQ], F32)
            nc.sync.dma_start(
                out=featt_sb, in_=featt_d[:].rearrange("(ec p) t -> p ec t", p=128)
            )

            ones_tq = cbf_sb[0:1, 0:TQ]
            bv_sb = cbf_sb[0:1, TQ : TQ + INNER]
            spb2_sb = cbf_sb[0:1, TQ + INNER : TQ + INNER + DH]
            bq_sb = cf32_sb[:, 0:8]    # per-head q bias, rows 0:64
            bk_sb = cf32_sb[:, 8:16]   # per-head k bias, rows 0:64
            spb1_sb = cf32_sb[:, 16:20]
            outb_sb = cf32_sb[:, 20:22]

            ident = constp.tile([128, 128], BF16)
            make_identity(nc, ident)
            eps_t = constp.tile([128, 1], F32)
            nc.vector.memset(eps_t, LN_EPS)

            # xyz|ones columns of Vaug (pre-scaled by 1/32 on host).
            # v and xyz parts live in separate tiles: the DoubleRow
            # stationary k-tile stride must be a power of two (walrus
            # rejects the fused 68-wide layout's stride of 544).
            vv_sb = bigp.tile([128, NT, H, DH], F8)
            vx_sb = bigp.tile([128, NT, H, 4], F8)
            for h in range(H):
                nc.gpsimd.tensor_copy(vx_sb[:, :, h, :], xyzv_sb)

            # PE priming: absorb one DMA-queue semaphore per DMA-loaded
            # tile the PE consumes + warm spam for the HAM clock gate.
            pwarm_cm = tc.tile_pool(name="pwarm", bufs=1, space="PSUM")
            pwarm = pwarm_cm.__enter__()
            warm_ps = pwarm.tile([128, 128], BF16, tag="warm", bufs=1)

            def warm(n):
                for _ in range(n):
                    nc.tensor.transpose(warm_ps, ident, ident)

            warm(24)
            prime_ps = pwarm.tile([4, 4], F32, tag="prime", bufs=1)

            def prime(lhsT, rhs):
                nc.tensor.matmul(
                    prime_ps[0 : lhsT.shape[-1], 0 : rhs.shape[-1]],
                    lhsT,
                    rhs,
                    start=True,
                    stop=True,
                )

            prime(wqkv_sb[:, 0, 0:4], wqkv_sb[:, 0, 0:4])
            prime(spw1_sb[:, 0:4], spw1_sb[:, 0:4])
            prime(spw2_sb[:, 0, 0:4], spw2_sb[:, 0, 0:4])
            prime(wout_sb[:, 0, 0:4], wout_sb[:, 0, 0:4])
            if has_bqkv:
                prime(ones_tq[:, 0:4], bv_sb[:, 0:4])
            if has_spb2:
                prime(spb2_sb[:, 0:4], ones_tq[:, 0:4])
            warm(12)
            pwarm_cm.__exit__(None, None, None)

            # ---- Phase A: LN -> transpose -> q/k/v, pipelined per 4-tile
            # group; LN stats for group g+1 issue ahead of group g's
            # evacuations so the ACT sqrt never queues behind them.
            # k is fully emitted here (attention needs all 8 PSUM banks).
            xn_sb = bigp.tile([128, NT, DIM], BF16)
            xnt_sb = bigp.tile([128, 2, M], F8)
            qt_sb = bigp.tile([64, H, TQ], BF16)
            kt_sb = bigp.tile([64, H, M], BF16)
            mv_all = constp.tile([128, NT, 2], F32)
            rstd = constp.tile([128, NT], F32)

            ptr_cm = tc.tile_pool(name="ptr", bufs=2, space="PSUM")
            ptr = ptr_cm.__enter__()
            pkq_cm = tc.tile_pool(name="pkq", bufs=2, space="PSUM")
            pkq = pkq_cm.__enter__()
            pv_cm = tc.tile_pool(name="pv", bufs=2, space="PSUM")
            pv = pv_cm.__enter__()

            def ln_stats(g):
                for q in range(4):
                    n = 4 * g + q
                    stats = workp.tile([128, 6], F32, tag="bnstats")
                    nc.vector.bn_stats(out=stats, in_=x_sb[:, n, :])
                    nc.vector.bn_aggr(out=mv_all[:, n, :], in_=stats)
                nc.scalar.activation(
                    out=rstd[:, 4 * g : 4 * g + 4],
                    in_=mv_all[:, 4 * g : 4 * g + 4, 1],
                    func=AF.Sqrt,
                    bias=eps_t,
                    scale=1.0,
                )

            def ln_recip(g):
                nc.vector.reciprocal(
                    out=rstd[:, 4 * g : 4 * g + 4],
                    in_=rstd[:, 4 * g : 4 * g + 4],
                )

            def emit_q():
                # 8 DoubleRow matmuls (one per head) over the DIM halves.
                for s in range(4):
                    ps_q = pkq.tile([64, 2, TQ], F32, tag="kq", bufs=2)
                    for oo in range(2):
                        h = 2 * s + oo
                        nc.tensor.matmul(
                            ps_q[:, oo, :],
                            wqkv_sb[:, :, h * 64 : (h + 1) * 64],
                            xnt_sb[:, :, 0:TQ],
                            start=True,
                            stop=True,
                            perf_mode=DR,
                        )
                    if has_bqkv:
                        for oo in range(2):
                            h = 2 * s + oo
                            nc.vector.tensor_scalar(
                                out=qt_sb[:, h, :],
                                in0=ps_q[:, oo, :],
                                scalar1=WQ_INV,
                                scalar2=bq_sb[0:64, h : h + 1],
                                op0=OP.mult,
                                op1=OP.add,
                            )
                    else:
                        nc.vector.tensor_scalar(
                            out=qt_sb[:, 2 * s : 2 * s + 2, :],
                            in0=ps_q,
                            scalar1=WQ_INV,
                            scalar2=None,
                            op0=OP.mult,
                        )

            def emit_k(g):
                # k for this group's 512 tokens, all 8 heads; evacuation
                # alternates DVE/ACT to balance the two engines.
                for s in range(4):
                    ps_k = pkq.tile([64, 2, TQ], F32, tag="kq", bufs=2)
                    for oo in range(2):
                        h = 2 * s + oo
                        base = INNER + h * 64
                        nc.tensor.matmul(
                            ps_k[:, oo, :],
                            wqkv_sb[:, :, base : base + 64],
                            xnt_sb[:, :, g * TQ : (g + 1) * TQ],
                            start=True,
                            stop=True,
                            perf_mode=DR,
                        )
                    dst = kt_sb[:, 2 * s : 2 * s + 2, g * TQ : (g + 1) * TQ]
                    if has_bqkv:
                        for oo in range(2):
                            h = 2 * s + oo
                            nc.vector.tensor_scalar(
                                out=kt_sb[:, h, g * TQ : (g + 1) * TQ],
                                in0=ps_k[:, oo, :],
                                scalar1=WQ_INV,
                                scalar2=bk_sb[0:64, h : h + 1],
                                op0=OP.mult,
                                op1=OP.add,
                            )
                    elif s % 2 == 0:
                        nc.scalar.activation(
                            out=dst, in_=ps_k, func=AF.Copy, scale=WQ_INV
                        )
                    else:
                        nc.vector.tensor_scalar(
                            out=dst,
                            in0=ps_k,
                            scalar1=WQ_INV,
                            scalar2=None,
                            op0=OP.mult,
                        )

            ln_stats(0)
            ln_recip(0)
            for g in range(4):
                if g + 1 < 4:
                    ln_stats(g + 1)
                for q in range(4):
                    n = 4 * g + q
                    nc.vector.tensor_scalar(
                        out=xn_sb[:, n, :],
                        in0=x_sb[:, n, :],
                        scalar1=mv_all[:, n, 0:1],
                        scalar2=rstd[:, n : n + 1],
                        op0=OP.subtract,
                        op1=OP.mult,
                    )
                # transpose this group into xnT (cast to fp8 at evac)
                for cc in range(2):
                    ps = ptr.tile([128, 512], BF16, tag="tr")
                    for q in range(4):
                        n = 4 * g + q
                        nc.tensor.transpose(
                            ps[:, q * 128 : (q + 1) * 128],
                            xn_sb[:, n, cc * 128 : (cc + 1) * 128],
                            ident,
                        )
                    nc.vector.tensor_copy(
                        xnt_sb[:, cc, g * 512 : (g + 1) * 512], ps
                    )
                if g + 1 < 4:
                    ln_recip(g + 1)
                if g == 0:
                    emit_q()
                emit_k(g)
                # v for this group: plain fp8 matmuls (out spans 128
                # token partitions), evacuated on the scalar engine.
                for q in range(4):
                    n = 4 * g + q
                    ps_v = pv.tile([128, INNER], F32, tag="v", bufs=2)
                    for cc in range(2):
                        nc.tensor.matmul(
                            ps_v,
                            xnt_sb[:, cc, n * 128 : (n + 1) * 128],
                            wqkv_sb[:, cc, 2 * INNER : 3 * INNER],
                            start=(cc == 0),
                            stop=(cc == 1 and not has_bqkv),
                        )
                    if has_bqkv:
                        nc.tensor.matmul(
                            ps_v,
                            ones_tq[:, 0:128],
                            bv_sb,
                            start=False,
                            stop=True,
                            skip_group_check=True,
                        )
                    nc.scalar.activation(
                        out=vv_sb[:, n, :, :],
                        in_=ps_v[:].rearrange("p (h d) -> p h d", h=H),
                        func=AF.Copy,
                        scale=WQ_INV,
                    )

            pv_cm.__exit__(None, None, None)
            pkq_cm.__exit__(None, None, None)
            ptr_cm.__exit__(None, None, None)

            # ---- attention: 4 passes x 2 heads, j-tiles processed in
            # pairs so AV runs as fp8 DoubleRow (2 j-tiles per matmul).
            # exp alternates: even tile -> scalar ACT (exact, fp8 out),
            # odd tile -> DVE Schraudolph into e4m3 bits.
            araw_sb = bigp.tile([64, 4, 2, TQ], F32)
            arax_sb = bigp.tile([4, 4, 2, TQ], F32)
            an_sb = bigp.tile([64, 4, 2, TQ], F32)
            dnp_sb = bigp.tile([3, 4, 2, TQ], BF16)
            rsp_cm = tc.tile_pool(name="rsp", bufs=2)
            rsp = rsp_cm.__enter__()
            with (
                tc.tile_pool(name="pattn", bufs=2, space="PSUM") as pattn,
                tc.tile_pool(name="expp", bufs=2) as expp,
            ):
                def qk_pair(p, j):
                    sT = pattn.tile([128, 2, TQ], F32, tag="sT", bufs=2)
                    for hh in range(2):
                        h = 2 * p + hh
                        nc.tensor.matmul(
                            sT[:, hh, :],
                            kt_sb[:, h, j * 128 : (j + 1) * 128],
                            qt_sb[:, h, :],
                            start=True,
                            stop=True,
                        )
                    return sT

                def exp_pair(sT0, sT1):
                    # e tile [128, jt, hh, i]; even tile on ACT, odd on DVE
                    e = expp.tile([128, 2, 2, TQ], F8, tag="e", bufs=2)
                    nc.scalar.activation(out=e[:, 0, :, :], in_=sT0, func=AF.Exp)
                    nc.vector.tensor_scalar(
                        out=e[:, 1, :, :].bitcast(I8),
                        in0=sT1,
                        scalar1=EXP_A8,
                        scalar2=EXP_B8,
                        op0=OP.mult,
                        op1=OP.add,
                    )
                    return e

                for p in range(4):
                    acc_v = pattn.tile([64, 2, TQ], F32, tag="accv", bufs=1)
                    acc_x = pattn.tile([4, 2, TQ], F32, tag="accx", bufs=1)
                    sT0, sT1 = qk_pair(p, 0), qk_pair(p, 1)
                    e_cur = exp_pair(sT0, sT1)
                    for t in range(8):
                        if t + 1 < 8:
                            sT0 = qk_pair(p, 2 * t + 2)
                            sT1 = qk_pair(p, 2 * t + 3)
                            e_nxt = exp_pair(sT0, sT1)
                        else:
                            e_nxt = None
                        for hh in range(2):
                            h = 2 * p + hh
                            nc.tensor.matmul(
                                acc_v[:, hh, :],
                                vv_sb[:, 2 * t : 2 * t + 2, h, :],
                                e_cur[:, :, hh, :],
                                start=(t == 0),
                                stop=(t == 7),
                                perf_mode=DR,
                            )
                            nc.tensor.matmul(
                                acc_x[:, hh, :],
                                vx_sb[:, 2 * t : 2 * t + 2, h, :],
                                e_cur[:, :, hh, :],
                                start=(t == 0),
                                stop=(t == 7),
                                perf_mode=DR,
                            )
                        e_cur = e_nxt
                    if p == 3:
                        # keep the PE busy through the norm chain + pool
                        # handoff so HAM stays at full clock into the MLP.
                        wps = pattn.tile([128, 2, TQ], F32, tag="sT", bufs=2)
                        wv = wps[:, 0, 0:64].bitcast(BF16)
                        for _ in range(20):
                            nc.tensor.transpose(wv, ident, ident)
                    nc.vector.tensor_copy(araw_sb[:, p, :, :], acc_v)
                    nc.vector.tensor_copy(arax_sb[:, p, :, :], acc_x)
                    # normalization runs under the next pass.  rbc holds
                    # 32/denominator (the ones column is 1/32), so an =
                    # 32*attn@v and the xyz rows (pre-scaled 1/32) come
                    # out at true scale.
                    rs = rsp.tile([128, 8], F32, tag="rs")
                    nc.sync.dma_start(out=rs, in_=arax_sb[3:4, p, :, :])
                    rc = rsp.tile([128, 8], F32, tag="rc")
                    nc.vector.reciprocal(out=rc, in_=rs)
                    rrow = rsp.tile([1, 2, TQ], F32, tag="rrow")
                    nc.sync.dma_start(out=rrow, in_=rc)
                    for hh in range(2):
                        rbc = rsp.tile([68, TQ], F32, tag="rbc", bufs=3)
                        nc.gpsimd.partition_broadcast(
                            rbc, rrow[0:1, hh, :], channels=68
                        )
                        nc.vector.tensor_tensor(
                            out=an_sb[:, p, hh, :],
                            in0=araw_sb[:, p, hh, :],
                            in1=rbc[0:64, :],
                            op=OP.mult,
                        )
                        dn = dnp_sb[:, p, hh, :]
                        nc.vector.tensor_tensor(
                            out=dn,
                            in0=arax_sb[0:3, p, hh, :],
                            in1=rbc[0:3, :],
                            op=OP.mult,
                        )
                        nc.vector.tensor_tensor(
                            out=dn,
                            in0=dn,
                            in1=xyzt_sb,
                            op=OP.subtract,
                        )
            rsp_cm.__exit__(None, None, None)

            # ---- spatial-bias MLP + out projection, pipelined per head:
            # h1 (bf16, kc pair) -> one gelu -> h2 (fp8 DoubleRow), then
            # outfin = an + sbias; out-proj (plain fp8) accumulates into
            # yT as soon as each head pair completes.
            outfin_sb = bigp.tile([64, H, TQ], F8)
            with (
                tc.tile_pool(name="pmlp", bufs=1, space="PSUM") as pmlp,
                tc.tile_pool(name="hpool", bufs=2) as hpool,
            ):
                yT = pmlp.tile([128, 2, TQ], F32, tag="yT", bufs=1)
                wv = yT[:, 0, 0:64].bitcast(BF16)
                for _ in range(10):
                    nc.tensor.transpose(wv, ident, ident)

                for m in range(4):
                    for hh in range(2):
                        h = 2 * m + hh
                        sb_t = pmlp.tile([64, TQ], F32, tag="sb", bufs=2)
                        for kcp in range(2):
                            h1 = pmlp.tile([128, 2, TQ], F32, tag="h1", bufs=2)
                            for kk in range(2):
                                kc = 2 * kcp + kk
                                nc.tensor.matmul(
                                    h1[:, kk, :],
                                    spw1_sb[:, kc * 128 : (kc + 1) * 128],
                                    dnp_sb[:, m, hh, :],
                                    start=True,
                                    stop=True,
                                )
                            hsb = hpool.tile([128, 2, TQ], F8, tag="hsb", bufs=2)
                            if has_spb1:
                                for kk in range(2):
                                    kc = 2 * kcp + kk
                                    nc.scalar.activation(
                                        out=hsb[:, kk, :],
                                        in_=h1[:, kk, :],
                                        func=AF.Gelu,
                                        bias=spb1_sb[:, kc : kc + 1],
                                    )
                            else:
                                nc.scalar.activation(
                                    out=hsb, in_=h1, func=AF.Gelu
                                )
                            nc.tensor.matmul(
                                sb_t,
                                spw2_sb[:, 2 * kcp : 2 * kcp + 2, :],
                                hsb,
                                start=(kcp == 0),
                                stop=(kcp == 1 and not has_spb2),
                                perf_mode=DR,
                            )
                        if has_spb2:
                            nc.tensor.matmul(
                                sb_t,
                                spb2_sb,
                                ones_tq,
                                start=False,
                                stop=True,
                                skip_group_check=True,
                            )
                        nc.vector.tensor_tensor(
                            out=outfin_sb[:, h, :],
                            in0=an_sb[:, m, hh, :],
                            in1=sb_t,
                            op=OP.add,
                        )
                    # out-projection contribution of this head pair
                    for hh in range(2):
                        h = 2 * m + hh
                        for ec in range(2):
                            nc.tensor.matmul(
                                yT[:, ec, :],
                                wout_sb[:, h, ec * 128 : (ec + 1) * 128],
                                outfin_sb[:, h, :],
                                start=(h == 0),
                                stop=(h == H - 1),
                            )

                # ---- final gelu (fused 2^-11 descale) + residual ----
                for ec in range(2):
                    ysb = workp.tile([128, TQ], F32, tag="ysb")
                    nc.scalar.activation(
                        out=ysb,
                        in_=yT[:, ec, :],
                        func=AF.Gelu,
                        bias=outb_sb[:, ec : ec + 1],
                        scale=YT_INV,
                    )
                    res = workp.tile([128, TQ], F32, tag="res")
                    nc.vector.tensor_tensor(
                        out=res, in0=ysb, in1=featt_sb[:, ec, :], op=OP.add
                    )
                    nc.sync.dma_start(
                        out=out_d[:].rearrange("(ec p) t -> p ec t", p=128)[:, ec, :],
                        in_=res,
                    )

    nc.compile()
    return nc


def prepare_maps(inputs):
    xyzs = np.asarray(inputs["xyzs"], np.float32)
    features = np.asarray(inputs["features"], np.float32)
    ln_g = np.asarray(inputs["ln_g"], np.float32)
    ln_b = np.asarray(inputs["ln_b"], np.float32)
    w_qkv = np.asarray(inputs["w_qkv"], np.float32)
    sp_w1 = np.asarray(inputs["sp_w1"], np.float32)
    sp_b1 = np.asarray(inputs["sp_b1"], np.float32)
    sp_w2 = np.asarray(inputs["sp_w2"], np.float32)
    sp_b2 = np.asarray(inputs["sp_b2"], np.float32)
    out_w = np.asarray(inputs["out_w"], np.float32)
    out_b = np.asarray(inputs["out_b"], np.float32)

    scale = DH ** -0.5
    wqkv_f = w_qkv * ln_g[:, None]
    wqkv_f[:, :INNER] = wqkv_f[:, :INNER] * scale
    bqkv = (ln_b @ w_qkv).astype(np.float32)
    bqkv[:INNER] *= scale

    has_bqkv = bool(np.any(bqkv != 0.0))
    has_spb1 = bool(np.any(sp_b1 != 0.0))
    has_spb2 = bool(np.any(sp_b2 != 0.0))

    cf32 = np.zeros((128, 24), np.float32)
    for h in range(H):
        cf32[0:64, h] = bqkv[h * 64 : (h + 1) * 64]
        cf32[0:64, 8 + h] = bqkv[INNER + h * 64 : INNER + (h + 1) * 64]
    for kc in range(4):
        cf32[:, 16 + kc] = sp_b1[kc * 128 : (kc + 1) * 128]
    cf32[:, 20] = out_b[:128]
    cf32[:, 21] = out_b[128:]

    cbf = np.zeros((1, TQ + INNER + DH), np.float32)
    cbf[0, 0:TQ] = 1.0
    cbf[0, TQ : TQ + INNER] = bqkv[2 * INNER :] * WQ_S
    cbf[0, TQ + INNER :] = sp_b2 * W2_S

    # wout as [64, H, 256]: row (d, h) = out_w[h*64+d, :]
    wout64 = np.ascontiguousarray(out_w.reshape(H, 64, DIM).transpose(1, 0, 2))

    shared = {
        "wqkv": np.ascontiguousarray(wqkv_f * WQ_S).astype(E4),
        "cf32": cf32,
        "cbf": cbf.astype(BF),
        "spw1": np.ascontiguousarray(sp_w1).astype(BF),
        "spw2": np.ascontiguousarray(sp_w2 * W2_S).astype(E4),
        "wout": (wout64 * WO_S).astype(E4),
    }

    in_maps = []
    for core in range(N_CORES):
        bi, quarter = core // 4, core % 4
        qs = quarter * TQ
        x_b = features[bi].reshape(M, DIM)
        xyz_b = xyzs[bi].reshape(M, 3)
        x_perm = np.roll(x_b, -qs, axis=0)
        xyz_perm = np.roll(xyz_b, -qs, axis=0)
        xyza = np.concatenate(
            [xyz_perm / OF_S, np.full((M, 1), 1.0 / OF_S, np.float32)], axis=1
        ).astype(np.float32)
        m = dict(shared)
        m["x"] = np.ascontiguousarray(x_perm).astype(BF)
        m["xyzv"] = np.ascontiguousarray(
            xyza.reshape(NT, 128, 4).transpose(1, 0, 2)
        ).astype(E4)
        m["xyzt"] = np.ascontiguousarray(xyz_perm[:TQ].T).astype(BF)
        m["featt"] = np.ascontiguousarray(x_perm[:TQ].T)
        in_maps.append(m)
    return in_maps, (has_bqkv, has_spb1, has_spb2)


def assemble(results, l=16, n=128):
    out = np.zeros((2, M, DIM), np.float32)
    for core in range(N_CORES):
        bi, quarter = core // 4, core % 4
        qs = quarter * TQ
        out[bi, qs : qs + TQ, :] = results[core]["out"].T
    return out.reshape(2, l, n, DIM)


def kernel(**inputs):
    in_maps, flags = prepare_maps(inputs)
    nc = build_program(*flags)
    results = run_bass_kernel_spmd(nc, in_maps, list(range(N_CORES))).results
    return assemble(results)


if __name__ == "__main__":
    pass


# revision 13
# speedup vs baseline: 1.3038x; 1.3038x over previous
"""Trainium2 Bass kernel for nn_Attention_1322849927460.

Dense transformer block: LN -> qkv -> attention (+ spatial-bias MLP on
attention-weighted coordinate deltas) -> out proj -> gelu -> residual.

Sharding: 8 cores = (2 batches) x (4 sequence quarters). Each core holds
all 8 heads for its 512 query rows and the full 2048-token K/V of its
batch, so no collectives are needed. A host-side roll of the token axis
puts each core's query rows first, letting all cores run an identical
SPMD program (attention is invariant to key-order permutation).

Algebraic structure:
  * delta_full[b,h,i,:] = (attn @ xyz)[b,h,i,:] - xyz[b,i,:] since softmax
    rows sum to one -> the (m,m,3) delta tensor is never formed.
  * softmax denominators come free from augmented V' columns [xyz/32, 1/32];
    one reciprocal + partition-broadcast normalizes the accumulators.  The
    1/32 ones-column also lands the normalized attention output at 32x
    true scale, lifting the fp8 outfin tensor out of e4m3 denormals free.
  * ln_g and the 1/sqrt(dh) q-scale fold into the qkv weights on host.

fp8 strategy (vs the bf16 baseline):
  * all projection matmuls run in fp8e4 (same column cost as bf16, but
    half the weight DMA); AV runs as fp8 DoubleRow over adjacent j-tile
    pairs (2 k-tiles per instruction, halving AV instruction count), with
    the xyz/ones columns as a second tiny DoubleRow into a base-0 [4, .]
    accumulator (DoubleRow outputs must start at partition 0, and the
    stationary k-tile stride must be a power of two - walrus crashes
    otherwise, hence split vv/vx tiles).  Spatial-MLP h2 is DoubleRow
    over kc pairs.  QK and MLP h1 stay bf16 (PSUM-output-bound).
  * weights quantized with power-of-2 host scales (wqkv x64, spw2 x32,
    wout x64), descaled for free inside evacuation ops / the final ACT.
  * exp alternates per j-tile: even tiles exact on the scalar engine,
    odd tiles on the DVE via Schraudolph into e4m3 bits.  Both write an
    int16-spaced e buffer (DVE needs a 2-byte output dtype for its 2x
    mode; i16 = round(x*8/ln2 + 55.66) has the e4m3 bit pattern in its
    low byte) and AV reads a stride-2 fp8 view.
"""

import os
import sys

for _p in ("/opt/trn_rl_repo",):
    if _p not in sys.path and os.path.isdir(_p):
        sys.path.insert(0, _p)

import ml_dtypes
import numpy as np

import concourse.bass as bass
import concourse.bacc as bacc
import concourse.tile as tile
from concourse import mybir
from concourse.bass_utils import run_bass_kernel_spmd
from concourse.masks import make_identity

F32 = mybir.dt.float32
BF16 = mybir.dt.bfloat16
F8 = mybir.dt.float8e4
I16 = mybir.dt.int16
AF = mybir.ActivationFunctionType
OP = mybir.AluOpType
DR = mybir.MatmulPerfMode.DoubleRow
BF = ml_dtypes.bfloat16
E4 = ml_dtypes.float8_e4m3

DIM = 256
H = 8
DH = 64
INNER = H * DH  # 512
M = 2048  # tokens per batch
TQ = 512  # query tokens per core
NT = M // 128  # 16 token tiles
N_CORES = 8
LN_EPS = 1e-5

# host-side fp8 weight scales (powers of two; descaled on-chip for free)
WQ_S = 64.0      # wqkv scale; descale 2^-6 in the q/k/v evacuations
W2_S = 32.0      # spw2 scale == the outfin x32 target scale
WO_S = 64.0      # wout scale; total descale 2^-11 in the final gelu
OF_S = 32.0      # outfin scale, produced by the 1/32 ones/xyz columns
WQ_INV = 1.0 / WQ_S
YT_INV = 1.0 / (OF_S * WO_S)

# Schraudolph fast exp in e4m3 bit domain:
#   e4m3(x) bits = round(x * 2^3/ln2 + (7*2^3 - 0.34)) viewed as int8.
EXP_A8 = 8.0 / float(np.log(2.0))
EXP_B8 = 56.0 - 0.34


def build_program(has_bqkv: bool, has_spb1: bool, has_spb2: bool):
    nc = bacc.Bacc()

    x_d = nc.dram_tensor("x", [M, DIM], BF16, kind="ExternalInput")
    xyzv_d = nc.dram_tensor("xyzv", [128, NT, 4], F8, kind="ExternalInput")
    xyzt_d = nc.dram_tensor("xyzt", [3, TQ], BF16, kind="ExternalInput")
    featt_d = nc.dram_tensor("featt", [DIM, TQ], F32, kind="ExternalInput")
    wqkv_d = nc.dram_tensor("wqkv", [DIM, 3 * INNER], F8, kind="ExternalInput")
    spw1_d = nc.dram_tensor("spw1", [3, 2 * DIM], BF16, kind="ExternalInput")
    spw2_d = nc.dram_tensor("spw2", [2 * DIM, DH], F8, kind="ExternalInput")
    wout_d = nc.dram_tensor("wout", [64, H, DIM], F8, kind="ExternalInput")
    cf32_d = nc.dram_tensor("cf32", [128, 16], F32, kind="ExternalInput")
    cbf_d = nc.dram_tensor("cbf", [1, TQ + INNER + DH], BF16, kind="ExternalInput")
    out_d = nc.dram_tensor("out", [DIM, TQ], F32, kind="ExternalOutput")

    with tile.TileContext(nc) as tc:
        with (
            tc.tile_pool(name="const", bufs=1) as constp,
            tc.tile_pool(name="big", bufs=1) as bigp,
            tc.tile_pool(name="work", bufs=2) as workp,
        ):
            # ---- DMAs: all on the sync HWDGE queue, critical-path first.
            wqkv_sb = constp.tile([128, 2, 3 * INNER], F8)
            nc.sync.dma_start(
                out=wqkv_sb, in_=wqkv_d[:].rearrange("(cc p) o -> p cc o", p=128)
            )
            x_sb = bigp.tile([128, NT, DIM], BF16)
            xv = x_d[:].rearrange("(n p) c -> p n c", p=128)
            for g in range(4):
                nc.sync.dma_start(
                    out=x_sb[:, 4 * g : 4 * g + 4, :],
                    in_=xv[:, 4 * g : 4 * g + 4, :],
                )
            xyzv_sb = constp.tile([128, NT, 4], F8)
            nc.sync.dma_start(out=xyzv_sb, in_=xyzv_d[:])
            xyzt_sb = constp.tile([3, TQ], BF16)
            nc.sync.dma_start(out=xyzt_sb, in_=xyzt_d[:])
            cbf_sb = constp.tile([1, TQ + INNER + DH], BF16)
            nc.sync.dma_start(out=cbf_sb, in_=cbf_d[:])
            cf32_sb = constp.tile([128, 16], F32)
            nc.sync.dma_start(out=cf32_sb, in_=cf32_d[:])
            spw1_sb = constp.tile([3, 2 * DIM], BF16)
            nc.sync.dma_start(out=spw1_sb, in_=spw1_d[:])
            spw2_sb = constp.tile([128, 4, DH], F8)
            nc.sync.dma_start(
                out=spw2_sb, in_=spw2_d[:].rearrange("(kc p) d -> p kc d", p=128)
            )
            wout_sb = constp.tile([64, H, DIM], F8)
            nc.sync.dma_start(out=wout_sb, in_=wout_d[:])
            featt_sb = constp.tile([128, 2, TQ], F32)
            nc.sync.dma_start(
                out=featt_sb, in_=featt_d[:].rearrange("(ec p) t -> p ec t", p=128)
            )

            ones_tq = cbf_sb[0:1, 0:TQ]
            bv_sb = cbf_sb[0:1, TQ : TQ + INNER]
            spb2_sb = cbf_sb[0:1, TQ + INNER : TQ + INNER + DH]
            bqk_sb = cf32_sb[:, 0:8]
            spb1_sb = cf32_sb[:, 8:12]
            outb_sb = cf32_sb[:, 12:14]

            ident = constp.tile([128, 128], BF16)
            make_identity(nc, ident)
            eps_t = constp.tile([128, 1], F32)
            nc.vector.memset(eps_t, LN_EPS)

            # xyz|ones columns of Vaug (pre-scaled by 1/32 on host); the
            # v and xyz parts live in separate tiles so each DoubleRow
            # stationary k-tile stride is a power of two.
            vv_sb = bigp.tile([128, NT, H, DH], F8)
            vx_sb = bigp.tile([128, NT, H, 4], F8)
            for h in range(H):
                nc.gpsimd.tensor_copy(vx_sb[:, :, h, :], xyzv_sb)

            # PE priming: absorb one DMA-queue semaphore per DMA-loaded
            # tile the PE consumes + warm spam for the HAM clock gate.
            pwarm_cm = tc.tile_pool(name="pwarm", bufs=1, space="PSUM")
            pwarm = pwarm_cm.__enter__()
            warm_ps = pwarm.tile([128, 128], BF16, tag="warm", bufs=1)

            def warm(n):
                for _ in range(n):
                    nc.tensor.transpose(warm_ps, ident, ident)

            warm(24)
            prime_ps = pwarm.tile([4, 4], F32, tag="prime", bufs=1)

            def prime(lhsT, rhs):
                nc.tensor.matmul(
                    prime_ps[0 : lhsT.shape[-1], 0 : rhs.shape[-1]],
                    lhsT,
                    rhs,
                    start=True,
                    stop=True,
                )

            prime(wqkv_sb[:, 0, 0:4], wqkv_sb[:, 0, 0:4])
            prime(spw1_sb[:, 0:4], spw1_sb[:, 0:4])
            prime(spw2_sb[:, 0, 0:4], spw2_sb[:, 0, 0:4])
            prime(wout_sb[:, 0, 0:4], wout_sb[:, 0, 0:4])
            if has_bqkv:
                prime(ones_tq[:, 0:4], bv_sb[:, 0:4])
            if has_spb2:
                prime(spb2_sb[:, 0:4], ones_tq[:, 0:4])
            warm(12)
            pwarm_cm.__exit__(None, None, None)

            # ---- Phase A: LN -> transpose -> q/k/v (plain fp8 matmuls),
            # pipelined per 4-tile group; LN stats for group g+1 issue
            # ahead of group g's evacuations so the ACT sqrt never queues
            # behind them.  All of k is emitted here (attention needs the
            # full 8 PSUM banks for itself).
            xn_sb = bigp.tile([128, NT, DIM], BF16)
            xnt_sb = bigp.tile([128, 2, M], F8)
            qt_sb = bigp.tile([128, 4, TQ], BF16)
            kt_sb = bigp.tile([128, 4, M], BF16)
            mv_all = constp.tile([128, NT, 2], F32)
            rstd = constp.tile([128, NT], F32)

            ptr_cm = tc.tile_pool(name="ptr", bufs=2, space="PSUM")
            ptr = ptr_cm.__enter__()
            pkq_cm = tc.tile_pool(name="pkq", bufs=2, space="PSUM")
            pkq = pkq_cm.__enter__()
            pv_cm = tc.tile_pool(name="pv", bufs=2, space="PSUM")
            pv = pv_cm.__enter__()

            def ln_stats(g):
                for q in range(4):
                    n = 4 * g + q
                    stats = workp.tile([128, 6], F32, tag="bnstats")
                    nc.vector.bn_stats(out=stats, in_=x_sb[:, n, :])
                    nc.vector.bn_aggr(out=mv_all[:, n, :], in_=stats)
                nc.scalar.activation(
                    out=rstd[:, 4 * g : 4 * g + 4],
                    in_=mv_all[:, 4 * g : 4 * g + 4, 1],
                    func=AF.Sqrt,
                    bias=eps_t,
                    scale=1.0,
                )

            def ln_recip(g):
                nc.vector.reciprocal(
                    out=rstd[:, 4 * g : 4 * g + 4],
                    in_=rstd[:, 4 * g : 4 * g + 4],
                )

            def emit_q():
                for grp in range(2):
                    ps_q = pkq.tile([128, 2, TQ], F32, tag="kq", bufs=2)
                    for oo in range(2):
                        oc = grp * 2 + oo
                        for cc in range(2):
                            nc.tensor.matmul(
                                ps_q[:, oo, :],
                                wqkv_sb[:, cc, oc * 128 : (oc + 1) * 128],
                                xnt_sb[:, cc, 0:TQ],
                                start=(cc == 0),
                                stop=(cc == 1),
                            )
                    if has_bqkv:
                        for oo in range(2):
                            oc = grp * 2 + oo
                            nc.vector.tensor_scalar(
                                out=qt_sb[:, oc, :],
                                in0=ps_q[:, oo, :],
                                scalar1=WQ_INV,
                                scalar2=bqk_sb[:, oc : oc + 1],
                                op0=OP.mult,
                                op1=OP.add,
                            )
                    else:
                        nc.vector.tensor_scalar(
                            out=qt_sb[:, grp * 2 : grp * 2 + 2, :],
                            in0=ps_q,
                            scalar1=WQ_INV,
                            scalar2=None,
                            op0=OP.mult,
                        )

            def emit_k(g):
                # k for this group's 512 tokens; evacuation alternates
                # ACT/DVE to balance the two engines.
                for grp in range(2):
                    ps_k = pkq.tile([128, 2, TQ], F32, tag="kq", bufs=2)
                    for oo in range(2):
                        oc = grp * 2 + oo
                        for cc in range(2):
                            nc.tensor.matmul(
                                ps_k[:, oo, :],
                                wqkv_sb[
                                    :, cc, INNER + oc * 128 : INNER + (oc + 1) * 128
                                ],
                                xnt_sb[:, cc, g * TQ : (g + 1) * TQ],
                                start=(cc == 0),
                                stop=(cc == 1),
                            )
                    dst = kt_sb[:, grp * 2 : grp * 2 + 2, g * TQ : (g + 1) * TQ]
                    if has_bqkv:
                        for oo in range(2):
                            oc = grp * 2 + oo
                            nc.vector.tensor_scalar(
                                out=kt_sb[:, oc, g * TQ : (g + 1) * TQ],
                                in0=ps_k[:, oo, :],
                                scalar1=WQ_INV,
                                scalar2=bqk_sb[:, 4 + oc : 5 + oc],
                                op0=OP.mult,
                                op1=OP.add,
                            )
                    elif grp == 0:
                        nc.scalar.activation(
                            out=dst, in_=ps_k, func=AF.Copy, scale=WQ_INV
                        )
                    else:
                        nc.vector.tensor_scalar(
                            out=dst,
                            in0=ps_k,
                            scalar1=WQ_INV,
                            scalar2=None,
                            op0=OP.mult,
                        )

            ln_stats(0)
            ln_recip(0)
            for g in range(4):
                if g + 1 < 4:
                    ln_stats(g + 1)
                for q in range(4):
                    n = 4 * g + q
                    nc.vector.tensor_scalar(
                        out=xn_sb[:, n, :],
                        in0=x_sb[:, n, :],
                        scalar1=mv_all[:, n, 0:1],
                        scalar2=rstd[:, n : n + 1],
                        op0=OP.subtract,
                        op1=OP.mult,
                    )
                # transpose this group into xnT (cast to fp8 at evac)
                for cc in range(2):
                    ps = ptr.tile([128, 512], BF16, tag="tr")
                    for q in range(4):
                        n = 4 * g + q
                        nc.tensor.transpose(
                            ps[:, q * 128 : (q + 1) * 128],
                            xn_sb[:, n, cc * 128 : (cc + 1) * 128],
                            ident,
                        )
                    nc.vector.tensor_copy(
                        xnt_sb[:, cc, g * 512 : (g + 1) * 512], ps
                    )
                if g + 1 < 4:
                    ln_recip(g + 1)
                if g == 0:
                    emit_q()
                emit_k(g)
                # v for this group, evacuated on the scalar engine.
                for q in range(4):
                    n = 4 * g + q
                    ps_v = pv.tile([128, INNER], F32, tag="v", bufs=2)
                    for cc in range(2):
                        nc.tensor.matmul(
                            ps_v,
                            xnt_sb[:, cc, n * 128 : (n + 1) * 128],
                            wqkv_sb[:, cc, 2 * INNER : 3 * INNER],
                            start=(cc == 0),
                            stop=(cc == 1 and not has_bqkv),
                        )
                    if has_bqkv:
                        nc.tensor.matmul(
                            ps_v,
                            ones_tq[:, 0:128],
                            bv_sb,
                            start=False,
                            stop=True,
                            skip_group_check=True,
                        )
                    nc.scalar.activation(
                        out=vv_sb[:, n, :, :],
                        in_=ps_v[:].rearrange("p (h d) -> p h d", h=H),
                        func=AF.Copy,
                        scale=WQ_INV,
                    )

            pv_cm.__exit__(None, None, None)
            pkq_cm.__exit__(None, None, None)
            ptr_cm.__exit__(None, None, None)

            # ---- attention: 4 passes x 2 heads, j-tiles processed in
            # pairs so AV runs as fp8 DoubleRow (2 j-tiles per matmul).
            araw_sb = bigp.tile([64, 4, 2, TQ], F32)
            arax_sb = bigp.tile([4, 4, 2, TQ], F32)
            an_sb = bigp.tile([64, 4, 2, TQ], F32)
            dnp_sb = bigp.tile([3, 4, 2, TQ], BF16)
            rsp_cm = tc.tile_pool(name="rsp", bufs=2)
            rsp = rsp_cm.__enter__()
            with (
                tc.tile_pool(name="pattn", bufs=2, space="PSUM") as pattn,
                tc.tile_pool(name="expp", bufs=2) as expp,
            ):
                def qk_pair(p, j):
                    sT = pattn.tile([128, 2, TQ], F32, tag="sT", bufs=2)
                    for hh in range(2):
                        nc.tensor.matmul(
                            sT[:, hh, :],
                            kt_sb[
                                hh * 64 : hh * 64 + 64,
                                p,
                                j * 128 : (j + 1) * 128,
                            ],
                            qt_sb[hh * 64 : hh * 64 + 64, p, :],
                            start=True,
                            stop=True,
                        )
                    return sT

                def exp_pair(sT0, sT1):
                    # int16-spaced e buffer [128, jt, hh, i]: ACT writes
                    # exact-exp fp8 into the low bytes of tile 0, the DVE
                    # writes Schraudolph i16 (low byte = e4m3 bits) into
                    # tile 1 at its 2x rate.  Returns the stride-2 fp8
                    # view that AV streams.
                    e = expp.tile([128, 2, 2, TQ], I16, tag="e", bufs=2)
                    e8 = e[:].bitcast(F8).rearrange(
                        "p a b (t two) -> p a b t two", two=2
                    )
                    nc.scalar.activation(
                        out=e8[:, 0, :, :, 0], in_=sT0, func=AF.Exp
                    )
                    nc.vector.tensor_scalar(
                        out=e[:, 1, :, :],
                        in0=sT1,
                        scalar1=EXP_A8,
                        scalar2=EXP_B8,
                        op0=OP.mult,
                        op1=OP.add,
                    )
                    return e8

                for p in range(4):
                    acc_v = pattn.tile([64, 2, TQ], F32, tag="accv", bufs=1)
                    acc_x = pattn.tile([4, 2, TQ], F32, tag="accx", bufs=1)
                    sT0, sT1 = qk_pair(p, 0), qk_pair(p, 1)
                    e_cur = exp_pair(sT0, sT1)
                    for t in range(8):
                        if t + 1 < 8:
                            sT0 = qk_pair(p, 2 * t + 2)
                            sT1 = qk_pair(p, 2 * t + 3)
                            e_nxt = exp_pair(sT0, sT1)
                        else:
                            e_nxt = None
                        for hh in range(2):
                            h = 2 * p + hh
                            nc.tensor.matmul(
                                acc_v[:, hh, :],
                                vv_sb[:, 2 * t : 2 * t + 2, h, :],
                                e_cur[:, :, hh, :, 0],
                                start=(t == 0),
                                stop=(t == 7),
                                perf_mode=DR,
                            )
                            nc.tensor.matmul(
                                acc_x[:, hh, :],
                                vx_sb[:, 2 * t : 2 * t + 2, h, :],
                                e_cur[:, :, hh, :, 0],
                                start=(t == 0),
                                stop=(t == 7),
                                perf_mode=DR,
                            )
                        e_cur = e_nxt
                    if p == 3:
                        # keep the PE busy through the norm chain + pool
                        # handoff so HAM stays at full clock into the MLP.
                        wps = pattn.tile([128, 2, TQ], F32, tag="sT", bufs=2)
                        wv = wps[:, 0, 0:64].bitcast(BF16)
                        for _ in range(20):
                            nc.tensor.transpose(wv, ident, ident)
                    nc.vector.tensor_copy(araw_sb[:, p, :, :], acc_v)
                    nc.vector.tensor_copy(arax_sb[:, p, :, :], acc_x)
                    # normalization runs under the next pass.  rbc holds
                    # 32/denominator (the ones column is 1/32), so an =
                    # 32*attn@v and the xyz rows (pre-scaled 1/32) come
                    # out at true scale.
                    rs = rsp.tile([128, 8], F32, tag="rs")
                    nc.sync.dma_start(out=rs, in_=arax_sb[3:4, p, :, :])
                    rc = rsp.tile([128, 8], F32, tag="rc")
                    nc.vector.reciprocal(out=rc, in_=rs)
                    rrow = rsp.tile([1, 2, TQ], F32, tag="rrow")
                    nc.sync.dma_start(out=rrow, in_=rc)
                    for hh in range(2):
                        rbc = rsp.tile([68, TQ], F32, tag="rbc", bufs=3)
                        nc.gpsimd.partition_broadcast(
                            rbc, rrow[0:1, hh, :], channels=68
                        )
                        nc.vector.tensor_tensor(
                            out=an_sb[:, p, hh, :],
                            in0=araw_sb[:, p, hh, :],
                            in1=rbc[0:64, :],
                            op=OP.mult,
                        )
                        dn = dnp_sb[:, p, hh, :]
                        nc.vector.tensor_tensor(
                            out=dn,
                            in0=arax_sb[0:3, p, hh, :],
                            in1=rbc[0:3, :],
                            op=OP.mult,
                        )
                        nc.vector.tensor_tensor(
                            out=dn,
                            in0=dn,
                            in1=xyzt_sb,
                            op=OP.subtract,
                        )
            rsp_cm.__exit__(None, None, None)

            # ---- spatial-bias MLP + out projection, pipelined per head:
            # h1 (bf16, kc pair) -> one gelu -> h2 (fp8 DoubleRow), then
            # outfin = an + sbias; out-proj (plain fp8) accumulates into
            # yT as soon as each head pair completes.
            outfin_sb = bigp.tile([64, H, TQ], F8)
            with (
                tc.tile_pool(name="pmlp", bufs=1, space="PSUM") as pmlp,
                tc.tile_pool(name="hpool", bufs=2) as hpool,
            ):
                yT = pmlp.tile([128, 2, TQ], F32, tag="yT", bufs=1)
                wv = yT[:, 0, 0:64].bitcast(BF16)
                for _ in range(10):
                    nc.tensor.transpose(wv, ident, ident)

                for m in range(4):
                    for hh in range(2):
                        h = 2 * m + hh
                        sb_t = pmlp.tile([64, TQ], F32, tag="sb", bufs=2)
                        for kcp in range(2):
                            h1 = pmlp.tile([128, 2, TQ], F32, tag="h1", bufs=2)
                            for kk in range(2):
                                kc = 2 * kcp + kk
                                nc.tensor.matmul(
                                    h1[:, kk, :],
                                    spw1_sb[:, kc * 128 : (kc + 1) * 128],
                                    dnp_sb[:, m, hh, :],
                                    start=True,
                                    stop=True,
                                )
                            hsb = hpool.tile([128, 2, TQ], F8, tag="hsb", bufs=2)
                            if has_spb1:
                                for kk in range(2):
                                    kc = 2 * kcp + kk
                                    nc.scalar.activation(
                                        out=hsb[:, kk, :],
                                        in_=h1[:, kk, :],
                                        func=AF.Gelu,
                                        bias=spb1_sb[:, kc : kc + 1],
                                    )
                            else:
                                nc.scalar.activation(
                                    out=hsb, in_=h1, func=AF.Gelu
                                )
                            nc.tensor.matmul(
                                sb_t,
                                spw2_sb[:, 2 * kcp : 2 * kcp + 2, :],
                                hsb,
                                start=(kcp == 0),
                                stop=(kcp == 1 and not has_spb2),
                                perf_mode=DR,
                            )
                        if has_spb2:
                            nc.tensor.matmul(
                                sb_t,
                                spb2_sb,
                                ones_tq,
                                start=False,
                                stop=True,
                                skip_group_check=True,
                            )
                        nc.vector.tensor_tensor(
                            out=outfin_sb[:, h, :],
                            in0=an_sb[:, m, hh, :],
                            in1=sb_t,
                            op=OP.add,
                        )
                    # out-projection contribution of this head pair
                    for hh in range(2):
                        h = 2 * m + hh
                        for ec in range(2):
                            nc.tensor.matmul(
                                yT[:, ec, :],
                                wout_sb[:, h, ec * 128 : (ec + 1) * 128],
                                outfin_sb[:, h, :],
                                start=(h == 0),
                                stop=(h == H - 1),
                            )

                # ---- final gelu (fused 2^-11 descale) + residual ----
                for ec in range(2):
                    ysb = workp.tile([128, TQ], F32, tag="ysb")
                    nc.scalar.activation(
                        out=ysb,
                        in_=yT[:, ec, :],
                        func=AF.Gelu,
                        bias=outb_sb[:, ec : ec + 1],
                        scale=YT_INV,
                    )
                    res = workp.tile([128, TQ], F32, tag="res")
                    nc.vector.tensor_tensor(
                        out=res, in0=ysb, in1=featt_sb[:, ec, :], op=OP.add
                    )
                    nc.sync.dma_start(
                        out=out_d[:].rearrange("(ec p) t -> p ec t", p=128)[:, ec, :],
                        in_=res,
                    )

    nc.compile()
    return nc


def prepare_maps(inputs):
    xyzs = np.asarray(inputs["xyzs"], np.float32)
    features = np.asarray(inputs["features"], np.float32)
    ln_g = np.asarray(inputs["ln_g"], np.float32)
    ln_b = np.asarray(inputs["ln_b"], np.float32)
    w_qkv = np.asarray(inputs["w_qkv"], np.float32)
    sp_w1 = np.asarray(inputs["sp_w1"], np.float32)
    sp_b1 = np.asarray(inputs["sp_b1"], np.float32)
    sp_w2 = np.asarray(inputs["sp_w2"], np.float32)
    sp_b2 = np.asarray(inputs["sp_b2"], np.float32)
    out_w = np.asarray(inputs["out_w"], np.float32)
    out_b = np.asarray(inputs["out_b"], np.float32)

    scale = DH ** -0.5
    wqkv_f = w_qkv * ln_g[:, None]
    wqkv_f[:, :INNER] = wqkv_f[:, :INNER] * scale
    bqkv = (ln_b @ w_qkv).astype(np.float32)
    bqkv[:INNER] *= scale

    has_bqkv = bool(np.any(bqkv != 0.0))
    has_spb1 = bool(np.any(sp_b1 != 0.0))
    has_spb2 = bool(np.any(sp_b2 != 0.0))

    cf32 = np.zeros((128, 16), np.float32)
    for oc in range(4):
        cf32[:, oc] = bqkv[oc * 128 : (oc + 1) * 128]
        cf32[:, 4 + oc] = bqkv[INNER + oc * 128 : INNER + (oc + 1) * 128]
    for kc in range(4):
        cf32[:, 8 + kc] = sp_b1[kc * 128 : (kc + 1) * 128]
    cf32[:, 12] = out_b[:128]
    cf32[:, 13] = out_b[128:]

    cbf = np.zeros((1, TQ + INNER + DH), np.float32)
    cbf[0, 0:TQ] = 1.0
    cbf[0, TQ : TQ + INNER] = bqkv[2 * INNER :] * WQ_S
    cbf[0, TQ + INNER :] = sp_b2 * W2_S

    # wout as [64, H, 256]: row (d, h) = out_w[h*64+d, :]
    wout64 = np.ascontiguousarray(out_w.reshape(H, 64, DIM).transpose(1, 0, 2))

    shared = {
        "wqkv": np.ascontiguousarray(wqkv_f * WQ_S).astype(E4),
        "cf32": cf32,
        "cbf": cbf.astype(BF),
        "spw1": np.ascontiguousarray(sp_w1).astype(BF),
        "spw2": np.ascontiguousarray(sp_w2 * W2_S).astype(E4),
        "wout": (wout64 * WO_S).astype(E4),
    }

    in_maps = []
    for core in range(N_CORES):
        bi, quarter = core // 4, core % 4
        qs = quarter * TQ
        x_b = features[bi].reshape(M, DIM)
        xyz_b = xyzs[bi].reshape(M, 3)
        x_perm = np.roll(x_b, -qs, axis=0)
        xyz_perm = np.roll(xyz_b, -qs, axis=0)
        xyza = np.concatenate(
            [xyz_perm / OF_S, np.full((M, 1), 1.0 / OF_S, np.float32)], axis=1
        ).astype(np.float32)
        m = dict(shared)
        m["x"] = np.ascontiguousarray(x_perm).astype(BF)
        m["xyzv"] = np.ascontiguousarray(
            xyza.reshape(NT, 128, 4).transpose(1, 0, 2)
        ).astype(E4)
        m["xyzt"] = np.ascontiguousarray(xyz_perm[:TQ].T).astype(BF)
        m["featt"] = np.ascontiguousarray(x_perm[:TQ].T)
        in_maps.append(m)
    return in_maps, (has_bqkv, has_spb1, has_spb2)


def assemble(results, l=16, n=128):
    out = np.zeros((2, M, DIM), np.float32)
    for core in range(N_CORES):
        bi, quarter = core // 4, core % 4
        qs = quarter * TQ
        out[bi, qs : qs + TQ, :] = results[core]["out"].T
    return out.reshape(2, l, n, DIM)


def kernel(**inputs):
    in_maps, flags = prepare_maps(inputs)
    nc = build_program(*flags)
    results = run_bass_kernel_spmd(nc, in_maps, list(range(N_CORES))).results
    return assemble(results)


if __name__ == "__main__":
    pass
